# revision 14
# baseline (speedup 1.0000x reference)
"""Trainium2 Bass kernel for geodesic convolution (gnn_message_passing).

Computation (per vertex v):
  x[v,ij,c]   = sum_t bary_w[v,ij,t] * signal[bary_idx[v,ij,t], c]
  conv[v,r,d] = sum_{ij,c} x[v,ij,c] * K[i(ij),(j(ij)+r)%NT,c,d]
  out[v,:]    = relu(conv[v, argmax_r ||conv[v,r,:]||, :])

Strategy: shard V across 8 cores. The signal lives resident in SBUF as an
fp16 pair table: table[p, e] = (sig[e, c(p)], sig[e+25000, c(p)]) packed in
one 32-bit word, which keeps ap_gather's num_elems under the 2^15 cap while
covering all 50000 rows. Slots are split across partition halves by ij
parity (partitions 0-63 even ij, 64-127 odd ij, channel = p%64) so the
matmul keeps a full K=128 contraction. Per 32-vertex quarter-tile, GPSIMD
ap_gather pulls one word per (ij,t,v) slot; host-prepared f32 weights carry
a zero in the wrong vertex-half lane, so the DVE multiply (f32, in place
over the weights) + tap/lane adds produce x in f32 channel-major. fp16 is
only used for the signal values themselves - everything downstream is f32,
which keeps the rotation-argmax flips rare enough for the 2e-2 gate.
20 accumulating f32 matmuls per 128-vertex tile read the rotated kernel
from a j-duplicated (rotation-deduped, parity-shifted) resident table, then
the norms/argmax/select/relu epilogue runs per tile.
"""

import numpy as np

# Problem constants (hardcoded; kernel.py must be self-contained).
V, NR, NT, CIN, COUT = 50000, 5, 8, 64, 64
NCORES = 8
VPC = V // NCORES            # 6250 vertices per core
TPT = 128                    # vertices per PE tile
NTILES = -(-VPC // TPT)      # 49
VPAD = NTILES * TPT          # 6272
HALFV = V // 2               # 25000 pair-table entries
IJ = NR * NT                 # 40
KP = IJ // 2                 # 20 ij-pairs (matmul chunks)
NQ = 4                       # gather quarters per PE tile
QT = TPT // NQ               # 32 vertices per quarter
NSLOTQ = KP * 3 * QT         # 1920 gather slots per partition-group
NWQ = NSLOTQ * 2             # 3840 weight lanes per group
NIWQ = NSLOTQ // 16          # 120 wrapped idx free dim
ND = NT * COUT               # 512 output cols (r,d)

_CACHE = {}


def build_program(ntiles=NTILES, repeat=1, gather_idxs=NSLOTQ, w_bcast=True):
    """Build the Bacc program for one SPMD core. Returns compiled nc.

    repeat > 1 duplicates the whole tile loop (same inputs/outputs) for
    wall-clock slope timing; the extra passes just overwrite the outputs.
    gather_idxs / w_bcast are timing-ablation knobs (default = real kernel).

    The f32 contraction runs as three accumulating bf16 matmul chains
    (x_hi*k_hi + x_hi*k_lo + x_lo*k_hi, with x = x_hi + x_lo an exact bf16
    split); the dropped x_lo*k_lo term is ~1e-5 relative. 3x512 columns at
    1 cyc/col beats 512 columns of 4-cyc f32.
    """
    import concourse.bass as bass
    import concourse.mybir as mybir
    import concourse.tile as tile
    from concourse import bacc

    f32 = mybir.dt.float32
    f16 = mybir.dt.float16
    i16 = mybir.dt.int16

    nc = bacc.Bacc(
        "TRN2",
        target_bir_lowering=False,
        debug=False,
        enable_asserts=False,
        num_devices=NCORES,
    )
    bf16 = mybir.dt.bfloat16
    sig_d = nc.dram_tensor("sigp", [128, HALFV, 2], f16, kind="ExternalInput")
    kdh_d = nc.dram_tensor("kdh", [128, NR, 16 * COUT], bf16, kind="ExternalInput")
    kdl_d = nc.dram_tensor("kdl", [128, NR, 16 * COUT], bf16, kind="ExternalInput")
    idx_d = nc.dram_tensor(
        "idx16", [ntiles * NQ * 128, NIWQ], i16, kind="ExternalInput"
    )
    wb_d = nc.dram_tensor("wb2", [ntiles, NQ, 2, NWQ], f32, kind="ExternalInput")
    out_d = nc.dram_tensor("out", [ntiles * TPT, COUT], f32, kind="ExternalOutput")

    with tile.TileContext(nc) as tc:
        with (
            tc.tile_pool(name="const", bufs=1) as cpool,
            tc.tile_pool(name="io", bufs=2) as iopool,
            tc.tile_pool(name="w", bufs=2) as wpool,
            tc.tile_pool(name="g", bufs=2) as gpool,
            tc.tile_pool(name="x", bufs=1) as xpool,
            tc.tile_pool(name="xs", bufs=2) as xspool,
            tc.tile_pool(name="epi", bufs=2) as epool,
            tc.tile_pool(name="ps", bufs=4, space="PSUM") as psA,
        ):
            table = cpool.tile([128, HALFV, 2], f16)
            nc.sync.dma_start(out=table[:], in_=sig_d.ap())
            kdh_t = cpool.tile([128, NR, 16 * COUT], bf16)
            nc.sync.dma_start(out=kdh_t[:], in_=kdh_d.ap())
            kdl_t = cpool.tile([128, NR, 16 * COUT], bf16)
            nc.sync.dma_start(out=kdl_t[:], in_=kdl_d.ap())

            for it_rep in range(ntiles * repeat):
                it = it_rep % ntiles
                x_t = xpool.tile([128, KP, TPT], f32, tag="x")
                i4_t = iopool.tile([128, NQ, NIWQ], i16, tag="i")
                nc.gpsimd.dma_start(
                    out=i4_t[:],
                    in_=idx_d.ap()[it * NQ * 128:(it + 1) * NQ * 128, :]
                    .rearrange("(q p) n -> p q n", p=128),
                )
                for q in range(NQ):
                    w_t = wpool.tile([128, NSLOTQ, 2], f32, tag="w")
                    wflat = w_t[:].rearrange("p n l -> p (n l)")
                    if w_bcast:
                        for parity in range(2):
                            nc.gpsimd.dma_start(
                                out=wflat[parity * 64:(parity + 1) * 64, :],
                                in_=wb_d.ap()[it, q, parity, :]
                                .unsqueeze(0)
                                .to_broadcast([64, NWQ]),
                            )
                    else:
                        nc.gpsimd.dma_start(
                            out=wflat[:, :],
                            in_=wb_d.ap()
                            .rearrange("t q l n -> (t q l) n")[0:128, :],
                        )
                    g_t = gpool.tile([128, NSLOTQ, 2], f16, tag="g")
                    nc.gpsimd.ap_gather(
                        out_ap=g_t[:],
                        in_ap=table[:],
                        idxs_ap=i4_t[:, q, :],
                        channels=128,
                        num_elems=HALFV,
                        d=2,
                        num_idxs=gather_idxs,
                    )
                    # xw = w * g in f32, in place over the weights.
                    gflat = g_t[:].rearrange("p n l -> p (n l)")
                    nc.vector.tensor_tensor(
                        out=wflat, in0=wflat, in1=gflat,
                        op=mybir.AluOpType.mult,
                    )
                    # Tap sum over t3 (in place into t=0 slot), then lane sum
                    # into the assembled x tile.
                    g5 = w_t[:].rearrange("p (k t v) l -> p k t v l", k=KP, t=3)
                    nc.vector.tensor_tensor(
                        out=g5[:, :, 0], in0=g5[:, :, 0], in1=g5[:, :, 1],
                        op=mybir.AluOpType.add,
                    )
                    nc.vector.tensor_tensor(
                        out=g5[:, :, 0], in0=g5[:, :, 0], in1=g5[:, :, 2],
                        op=mybir.AluOpType.add,
                    )
                    nc.vector.tensor_tensor(
                        out=x_t[:, :, q * QT:(q + 1) * QT],
                        in0=g5[:, :, 0, :, 0],
                        in1=g5[:, :, 0, :, 1],
                        op=mybir.AluOpType.add,
                    )

                # Exact bf16 split of x; the matmul runs as three bf16
                # chains, dropping only the ~1e-5 x_lo*k_lo term.
                xh_t = xspool.tile([128, KP, TPT], bf16, tag="xh")
                nc.scalar.copy(out=xh_t[:], in_=x_t[:])
                xl_t = xspool.tile([128, KP, TPT], bf16, tag="xl")
                nc.vector.tensor_tensor(
                    out=xl_t[:], in0=x_t[:], in1=xh_t[:],
                    op=mybir.AluOpType.subtract,
                )
                conv_p = psA.tile([128, ND], f32, tag="conv")
                chains = ((xh_t, kdh_t), (xh_t, kdl_t), (xl_t, kdh_t))
                for ci, (xs_t, kd_t) in enumerate(chains):
                    for k in range(KP):
                        j0 = (2 * k) % NT
                        i0 = (2 * k) // NT
                        nc.tensor.matmul(
                            conv_p[:],
                            lhsT=xs_t[:, k, :],
                            rhs=kd_t[:, i0, j0 * COUT:j0 * COUT + ND],
                            start=(ci == 0 and k == 0),
                            stop=(ci == 2 and k == KP - 1),
                        )

                # Epilogue: norms over d, argmax over r (via is_equal mask),
                # masked-sum select, relu.
                sq_t = epool.tile([128, ND], f32, tag="sq")
                nc.scalar.activation(
                    out=sq_t[:], in_=conv_p[:],
                    func=mybir.ActivationFunctionType.Square,
                )
                norm_t = epool.tile([128, NT], f32, tag="norm")
                nc.vector.tensor_reduce(
                    out=norm_t[:],
                    in_=sq_t[:].rearrange("p (r d) -> p r d", d=COUT),
                    axis=mybir.AxisListType.X,
                    op=mybir.AluOpType.add,
                )
                mx_t = epool.tile([128, 1], f32, tag="mx")
                nc.vector.tensor_reduce(
                    out=mx_t[:], in_=norm_t[:],
                    axis=mybir.AxisListType.X, op=mybir.AluOpType.max,
                )
                mask_t = epool.tile([128, NT], f32, tag="mask")
                nc.vector.tensor_scalar(
                    out=mask_t[:], in0=norm_t[:], scalar1=mx_t[:], scalar2=None,
                    op0=mybir.AluOpType.is_equal,
                )
                msel_t = epool.tile([128, NT, COUT], f32, tag="sq")
                nc.vector.tensor_tensor(
                    out=msel_t[:],
                    in0=conv_p[:].rearrange("p (r d) -> p r d", d=COUT),
                    in1=mask_t[:].unsqueeze(-1).to_broadcast([128, NT, COUT]),
                    op=mybir.AluOpType.mult,
                )
                o_t = epool.tile([128, COUT], f32, tag="o")
                nc.vector.tensor_reduce(
                    out=o_t[:],
                    in_=msel_t[:].rearrange("p r d -> p d r"),
                    axis=mybir.AxisListType.X,
                    op=mybir.AluOpType.add,
                )
                nc.vector.tensor_scalar_max(o_t[:], o_t[:], 0.0)
                nc.sync.dma_start(
                    out=out_d.ap()[it * TPT:(it + 1) * TPT, :], in_=o_t[:]
                )

    nc.compile()
    return nc


def _host_prep(signal, bary_w, bary_idx, kernel, ntiles=NTILES):
    """Build per-core input maps. All host-side numpy, not timed."""
    import ml_dtypes

    bf16 = ml_dtypes.bfloat16
    kern = np.asarray(kernel, np.float32)

    # j-duplicated, parity-shifted rotated kernel:
    # kdup[p, i, jj*64+d] = K[i, (jj + p//64) % NT, p%64, d], jj in [0,16).
    jj16 = np.arange(16)
    parts = []
    for par in range(2):
        kdp = kern[:, (jj16 + par) % NT, :, :]        # [i, jj, c, d]
        parts.append(kdp.transpose(2, 0, 1, 3))        # [c, i, jj, d]
    kdup = np.ascontiguousarray(
        np.concatenate(parts, axis=0).reshape(128, NR, 16 * COUT)
    )
    kdh = kdup.astype(bf16)
    kdl = (kdup - kdh.astype(np.float32)).astype(bf16)

    # Signal pair table: table[p, e] = (sig[e, c], sig[e+HALFV, c]), c = p%64.
    sb = np.asarray(signal).astype(np.float16)         # [V, 64]
    sigp = np.empty((128, HALFV, 2), np.float16)
    sigp[:, :, 0] = np.tile(sb[:HALFV].T, (2, 1))
    sigp[:, :, 1] = np.tile(sb[HALFV:].T, (2, 1))

    vpad = ntiles * TPT
    idxf = np.asarray(bary_idx).reshape(V, IJ, 3)
    wff = np.asarray(bary_w).reshape(V, IJ, 3).astype(np.float32)
    in_maps = []
    for c in range(NCORES):
        sl = slice(c * VPC, (c + 1) * VPC)
        n = min(VPC, vpad)
        idx = np.zeros((vpad, IJ, 3), np.int32)
        idx[:n] = idxf[sl][:n]
        w = np.zeros((vpad, IJ, 3), np.float32)
        w[:n] = wff[sl][:n]
        # [t, q, v32, k, parity, t3] -> [t, q, parity, k, t3, v32]
        idx_r = idx.reshape(ntiles, NQ, QT, KP, 2, 3).transpose(0, 1, 4, 3, 5, 2)
        w_r = w.reshape(ntiles, NQ, QT, KP, 2, 3).transpose(0, 1, 4, 3, 5, 2)
        e = (idx_r % HALFV).astype(np.int16)
        lane = idx_r // HALFV
        wb2 = np.stack(
            [w_r * (lane == 0), w_r * (lane == 1)], axis=-1
        )                                              # [t, q, 2, k, 3, v32, 2]
        wb2 = np.ascontiguousarray(wb2.reshape(ntiles, NQ, 2, NWQ), np.float32)
        # Wrapped idx: list position i at [i%16, i//16]; 4 copies per half.
        el = e.reshape(ntiles, NQ, 2, NIWQ, 16)
        wr = np.swapaxes(el, -1, -2)                   # [t, q, parity, 16, NIWQ]
        idx16 = np.empty((ntiles, NQ, 128, NIWQ), np.int16)
        idx16[:, :, 0:64] = np.tile(wr[:, :, 0], (1, 1, 4, 1))
        idx16[:, :, 64:128] = np.tile(wr[:, :, 1], (1, 1, 4, 1))
        in_maps.append({
            "sigp": sigp,
            "kdh": kdh,
            "kdl": kdl,
            "idx16": np.ascontiguousarray(
                idx16.reshape(ntiles * NQ * 128, NIWQ)
            ),
            "wb2": wb2,
        })
    return in_maps


def kernel(signal, bary_w, bary_idx, kernel):
    from concourse.bass_utils import run_bass_kernel_spmd

    if "nc" not in _CACHE:
        _CACHE["nc"] = build_program()
    nc = _CACHE["nc"]
    in_maps = _host_prep(signal, bary_w, bary_idx, kernel)
    res = run_bass_kernel_spmd(nc, in_maps, core_ids=list(range(NCORES)))
    out = np.concatenate(
        [res.results[c]["out"][:VPC] for c in range(NCORES)], axis=0
    )
    return out.astype(np.float32)


# revision 18
# speedup vs baseline: 1.0616x; 1.0616x over previous
"""Trainium2 Bass kernel for geodesic convolution (gnn_message_passing).

Computation (per vertex v):
  x[v,ij,c]   = sum_t bary_w[v,ij,t] * signal[bary_idx[v,ij,t], c]
  conv[v,r,d] = sum_{ij,c} x[v,ij,c] * K[i(ij),(j(ij)+r)%NT,c,d]
  out[v,:]    = relu(conv[v, argmax_r ||conv[v,r,:]||, :])

Strategy: shard V across 8 cores. The signal lives resident in SBUF as an
fp16 pair table: table[p, e] = (sig[e, c(p)], sig[e+25000, c(p)]) packed in
one 32-bit word, which keeps ap_gather's num_elems under the 2^15 cap while
covering all 50000 rows. Slots are split across partition halves by ij
parity (partitions 0-63 even ij, 64-127 odd ij, channel = p%64) so the
matmul keeps a full K=128 contraction. Per 32-vertex quarter-tile, GPSIMD
ap_gather pulls one word per (ij,t,v) slot; host-prepared f32 weights carry
a zero in the wrong vertex-half lane, so the DVE multiply (f32, in place
over the weights) + tap/lane adds produce x in f32 channel-major. fp16 is
only used for the signal values themselves - everything downstream is f32,
which keeps the rotation-argmax flips rare enough for the 2e-2 gate.
20 accumulating f32 matmuls per 128-vertex tile read the rotated kernel
from a j-duplicated (rotation-deduped, parity-shifted) resident table, then
the norms/argmax/select/relu epilogue runs per tile.
"""

import numpy as np

# Problem constants (hardcoded; kernel.py must be self-contained).
V, NR, NT, CIN, COUT = 50000, 5, 8, 64, 64
NCORES = 8
VPC = V // NCORES            # 6250 vertices per core
TPT = 128                    # vertices per PE tile
NTILES = -(-VPC // TPT)      # 49
VPAD = NTILES * TPT          # 6272
HALFV = V // 2               # 25000 pair-table entries
IJ = NR * NT                 # 40
KP = IJ // 2                 # 20 ij-pairs (matmul chunks)
NQ = 4                       # gather quarters per PE tile
QT = TPT // NQ               # 32 vertices per quarter
NSLOTQ = KP * 3 * QT         # 1920 gather slots per partition-group
NWQ = NSLOTQ * 2             # 3840 weight lanes per group
NIWQ = NSLOTQ // 16          # 120 wrapped idx free dim
ND = NT * COUT               # 512 output cols (r,d)

_CACHE = {}


def build_program(ntiles=NTILES, repeat=1, gather_idxs=NSLOTQ, w_bcast=True):
    """Build the Bacc program for one SPMD core. Returns compiled nc.

    repeat > 1 duplicates the whole tile loop (same inputs/outputs) for
    wall-clock slope timing; the extra passes just overwrite the outputs.
    gather_idxs / w_bcast are timing-ablation knobs (default = real kernel).

    The f32 contraction runs as three accumulating bf16 matmul chains
    (x_hi*k_hi + x_hi*k_lo + x_lo*k_hi, with x = x_hi + x_lo an exact bf16
    split); the dropped x_lo*k_lo term is ~1e-5 relative. 3x512 columns at
    1 cyc/col beats 512 columns of 4-cyc f32.
    """
    import concourse.bass as bass
    import concourse.mybir as mybir
    import concourse.tile as tile
    from concourse import bacc

    f32 = mybir.dt.float32
    f16 = mybir.dt.float16
    i16 = mybir.dt.int16

    nc = bacc.Bacc(
        "TRN2",
        target_bir_lowering=False,
        debug=False,
        enable_asserts=False,
        num_devices=NCORES,
    )
    bf16 = mybir.dt.bfloat16
    sig_d = nc.dram_tensor("sigp", [128, HALFV, 2], f16, kind="ExternalInput")
    kdh_d = nc.dram_tensor("kdh", [128, NR, 16 * COUT], bf16, kind="ExternalInput")
    kdl_d = nc.dram_tensor("kdl", [128, NR, 16 * COUT], bf16, kind="ExternalInput")
    idx_d = nc.dram_tensor(
        "idx16", [ntiles * NQ * 128, NIWQ], i16, kind="ExternalInput"
    )
    wb_d = nc.dram_tensor("wb2", [ntiles, NQ, 2, NSLOTQ], f32, kind="ExternalInput")
    mk_d = nc.dram_tensor(
        "mask8", [ntiles, NQ, 2, NSLOTQ], mybir.dt.uint8, kind="ExternalInput"
    )
    out_d = nc.dram_tensor("out", [ntiles * TPT, COUT], f32, kind="ExternalOutput")

    with tile.TileContext(nc) as tc:
        with (
            tc.tile_pool(name="const", bufs=1) as cpool,
            tc.tile_pool(name="io", bufs=2) as iopool,
            tc.tile_pool(name="w", bufs=2) as wpool,
            tc.tile_pool(name="g", bufs=2) as gpool,
            tc.tile_pool(name="x", bufs=1) as xpool,
            tc.tile_pool(name="xs", bufs=2) as xspool,
            tc.tile_pool(name="epi", bufs=2) as epool,
            tc.tile_pool(name="ps", bufs=4, space="PSUM") as psA,
        ):
            table = cpool.tile([128, HALFV, 2], f16)
            nc.sync.dma_start(out=table[:], in_=sig_d.ap())
            kdh_t = cpool.tile([128, NR, 16 * COUT], bf16)
            nc.sync.dma_start(out=kdh_t[:], in_=kdh_d.ap())
            kdl_t = cpool.tile([128, NR, 16 * COUT], bf16)
            nc.sync.dma_start(out=kdl_t[:], in_=kdl_d.ap())

            for it_rep in range(ntiles * repeat):
                it = it_rep % ntiles
                x_t = xpool.tile([128, KP, TPT], f32, tag="x")
                i4_t = iopool.tile([128, NQ, NIWQ], i16, tag="i")
                nc.gpsimd.dma_start(
                    out=i4_t[:],
                    in_=idx_d.ap()[it * NQ * 128:(it + 1) * NQ * 128, :]
                    .rearrange("(q p) n -> p q n", p=128),
                )
                for q in range(NQ):
                    w_t = wpool.tile([128, NSLOTQ], f32, tag="w")
                    m_t = wpool.tile([128, NSLOTQ], mybir.dt.uint8, tag="m")
                    for parity in range(2):
                        rows = slice(parity * 64, (parity + 1) * 64)
                        nc.gpsimd.dma_start(
                            out=w_t[rows, :],
                            in_=wb_d.ap()[it, q, parity, :]
                            .unsqueeze(0)
                            .to_broadcast([64, NSLOTQ]),
                        )
                        nc.gpsimd.dma_start(
                            out=m_t[rows, :],
                            in_=mk_d.ap()[it, q, parity, :]
                            .unsqueeze(0)
                            .to_broadcast([64, NSLOTQ]),
                        )
                    g_t = gpool.tile([128, NSLOTQ, 2], f16, tag="g")
                    nc.gpsimd.ap_gather(
                        out_ap=g_t[:],
                        in_ap=table[:],
                        idxs_ap=i4_t[:, q, :],
                        channels=128,
                        num_elems=HALFV,
                        d=2,
                        num_idxs=gather_idxs,
                    )
                    # Lane merge: g[:, :, 0] = mask ? g[:, :, 1] : g[:, :, 0]
                    nc.vector.copy_predicated(
                        out=g_t[:, :, 0], mask=m_t[:], data=g_t[:, :, 1]
                    )
                    # xw = w * g_selected in f32, in place over the weights.
                    nc.vector.tensor_tensor(
                        out=w_t[:], in0=w_t[:], in1=g_t[:, :, 0],
                        op=mybir.AluOpType.mult,
                    )
                    # Tap sum over t3 (in place into t=0 slot), second add
                    # lands in the assembled x tile.
                    w4 = w_t[:].rearrange("p (k t v) -> p k t v", k=KP, t=3)
                    nc.vector.tensor_tensor(
                        out=w4[:, :, 0], in0=w4[:, :, 0], in1=w4[:, :, 1],
                        op=mybir.AluOpType.add,
                    )
                    nc.vector.tensor_tensor(
                        out=x_t[:, :, q * QT:(q + 1) * QT],
                        in0=w4[:, :, 0], in1=w4[:, :, 2],
                        op=mybir.AluOpType.add,
                    )

                # Exact bf16 split of x; the matmul runs as three bf16
                # chains, dropping only the ~1e-5 x_lo*k_lo term.
                xh_t = xspool.tile([128, KP, TPT], bf16, tag="xh")
                nc.scalar.copy(out=xh_t[:], in_=x_t[:])
                xl_t = xspool.tile([128, KP, TPT], bf16, tag="xl")
                nc.vector.tensor_tensor(
                    out=xl_t[:], in0=x_t[:], in1=xh_t[:],
                    op=mybir.AluOpType.subtract,
                )
                conv_p = psA.tile([128, ND], f32, tag="conv")
                chains = ((xh_t, kdh_t), (xh_t, kdl_t), (xl_t, kdh_t))
                for ci, (xs_t, kd_t) in enumerate(chains):
                    for k in range(KP):
                        j0 = (2 * k) % NT
                        i0 = (2 * k) // NT
                        nc.tensor.matmul(
                            conv_p[:],
                            lhsT=xs_t[:, k, :],
                            rhs=kd_t[:, i0, j0 * COUT:j0 * COUT + ND],
                            start=(ci == 0 and k == 0),
                            stop=(ci == 2 and k == KP - 1),
                        )

                # Epilogue: norms over d, argmax over r (via is_equal mask),
                # masked-sum select, relu.
                sq_t = epool.tile([128, ND], f32, tag="sq")
                nc.scalar.activation(
                    out=sq_t[:], in_=conv_p[:],
                    func=mybir.ActivationFunctionType.Square,
                )
                norm_t = epool.tile([128, NT], f32, tag="norm")
                nc.vector.tensor_reduce(
                    out=norm_t[:],
                    in_=sq_t[:].rearrange("p (r d) -> p r d", d=COUT),
                    axis=mybir.AxisListType.X,
                    op=mybir.AluOpType.add,
                )
                mx_t = epool.tile([128, 1], f32, tag="mx")
                nc.vector.tensor_reduce(
                    out=mx_t[:], in_=norm_t[:],
                    axis=mybir.AxisListType.X, op=mybir.AluOpType.max,
                )
                mask_t = epool.tile([128, NT], f32, tag="mask")
                nc.vector.tensor_scalar(
                    out=mask_t[:], in0=norm_t[:], scalar1=mx_t[:], scalar2=None,
                    op0=mybir.AluOpType.is_equal,
                )
                msel_t = epool.tile([128, NT, COUT], f32, tag="sq")
                nc.vector.tensor_tensor(
                    out=msel_t[:],
                    in0=conv_p[:].rearrange("p (r d) -> p r d", d=COUT),
                    in1=mask_t[:].unsqueeze(-1).to_broadcast([128, NT, COUT]),
                    op=mybir.AluOpType.mult,
                )
                o_t = epool.tile([128, COUT], f32, tag="o")
                nc.vector.tensor_reduce(
                    out=o_t[:],
                    in_=msel_t[:].rearrange("p r d -> p d r"),
                    axis=mybir.AxisListType.X,
                    op=mybir.AluOpType.add,
                )
                nc.vector.tensor_scalar_max(o_t[:], o_t[:], 0.0)
                nc.sync.dma_start(
                    out=out_d.ap()[it * TPT:(it + 1) * TPT, :], in_=o_t[:]
                )

    nc.compile()
    return nc


def _host_prep(signal, bary_w, bary_idx, kernel, ntiles=NTILES):
    """Build per-core input maps. All host-side numpy, not timed."""
    import ml_dtypes

    bf16 = ml_dtypes.bfloat16
    kern = np.asarray(kernel, np.float32)

    # j-duplicated, parity-shifted rotated kernel:
    # kdup[p, i, jj*64+d] = K[i, (jj + p//64) % NT, p%64, d], jj in [0,16).
    jj16 = np.arange(16)
    parts = []
    for par in range(2):
        kdp = kern[:, (jj16 + par) % NT, :, :]        # [i, jj, c, d]
        parts.append(kdp.transpose(2, 0, 1, 3))        # [c, i, jj, d]
    kdup = np.ascontiguousarray(
        np.concatenate(parts, axis=0).reshape(128, NR, 16 * COUT)
    )
    kdh = kdup.astype(bf16)
    kdl = (kdup - kdh.astype(np.float32)).astype(bf16)

    # Signal pair table: table[p, e] = (sig[e, c], sig[e+HALFV, c]), c = p%64.
    sb = np.asarray(signal).astype(np.float16)         # [V, 64]
    sigp = np.empty((128, HALFV, 2), np.float16)
    sigp[:, :, 0] = np.tile(sb[:HALFV].T, (2, 1))
    sigp[:, :, 1] = np.tile(sb[HALFV:].T, (2, 1))

    vpad = ntiles * TPT
    idxf = np.asarray(bary_idx).reshape(V, IJ, 3)
    wff = np.asarray(bary_w).reshape(V, IJ, 3).astype(np.float32)
    in_maps = []
    for c in range(NCORES):
        sl = slice(c * VPC, (c + 1) * VPC)
        n = min(VPC, vpad)
        idx = np.zeros((vpad, IJ, 3), np.int32)
        idx[:n] = idxf[sl][:n]
        w = np.zeros((vpad, IJ, 3), np.float32)
        w[:n] = wff[sl][:n]
        # [t, q, v32, k, parity, t3] -> [t, q, parity, k, t3, v32]
        idx_r = idx.reshape(ntiles, NQ, QT, KP, 2, 3).transpose(0, 1, 4, 3, 5, 2)
        w_r = w.reshape(ntiles, NQ, QT, KP, 2, 3).transpose(0, 1, 4, 3, 5, 2)
        e = (idx_r % HALFV).astype(np.int16)
        lane = idx_r // HALFV
        wb2 = np.ascontiguousarray(
            w_r.reshape(ntiles, NQ, 2, NSLOTQ), np.float32
        )
        mask8 = np.ascontiguousarray(
            lane.astype(np.uint8).reshape(ntiles, NQ, 2, NSLOTQ)
        )
        # Wrapped idx: list position i at [i%16, i//16]; 4 copies per half.
        el = e.reshape(ntiles, NQ, 2, NIWQ, 16)
        wr = np.swapaxes(el, -1, -2)                   # [t, q, parity, 16, NIWQ]
        idx16 = np.empty((ntiles, NQ, 128, NIWQ), np.int16)
        idx16[:, :, 0:64] = np.tile(wr[:, :, 0], (1, 1, 4, 1))
        idx16[:, :, 64:128] = np.tile(wr[:, :, 1], (1, 1, 4, 1))
        in_maps.append({
            "sigp": sigp,
            "kdh": kdh,
            "kdl": kdl,
            "idx16": np.ascontiguousarray(
                idx16.reshape(ntiles * NQ * 128, NIWQ)
            ),
            "wb2": wb2,
            "mask8": mask8,
        })
    return in_maps


def kernel(signal, bary_w, bary_idx, kernel):
    from concourse.bass_utils import run_bass_kernel_spmd

    if "nc" not in _CACHE:
        _CACHE["nc"] = build_program()
    nc = _CACHE["nc"]
    in_maps = _host_prep(signal, bary_w, bary_idx, kernel)
    res = run_bass_kernel_spmd(nc, in_maps, core_ids=list(range(NCORES)))
    out = np.concatenate(
        [res.results[c]["out"][:VPC] for c in range(NCORES)], axis=0
    )
    return out.astype(np.float32)


# revision 20
# speedup vs baseline: 1.6770x; 1.5797x over previous
"""Trainium2 Bass kernel for geodesic convolution (gnn_message_passing).

Computation (per vertex v):
  x[v,ij,c]   = sum_t bary_w[v,ij,t] * signal[bary_idx[v,ij,t], c]
  conv[v,r,d] = sum_{ij,c} x[v,ij,c] * K[i(ij),(j(ij)+r)%NT,c,d]
  out[v,:]    = relu(conv[v, argmax_r ||conv[v,r,:]||, :])

Strategy: shard V across 8 cores. The signal lives resident in SBUF as an
fp16 pair table: table[p, e] = (sig[e, c(p)], sig[e+25000, c(p)]) packed in
one 32-bit word, which keeps ap_gather's num_elems under the 2^15 cap while
covering all 50000 rows. Slots are split across partition halves by ij
parity (partitions 0-63 even ij, 64-127 odd ij, channel = p%64) so the
matmul keeps a full K=128 contraction. Per 32-vertex quarter-tile, GPSIMD
ap_gather pulls one word per (ij,t,v) slot; host-prepared f32 weights carry
a zero in the wrong vertex-half lane, so the DVE multiply (f32, in place
over the weights) + tap/lane adds produce x in f32 channel-major. fp16 is
only used for the signal values themselves - everything downstream is f32,
which keeps the rotation-argmax flips rare enough for the 2e-2 gate.
20 accumulating f32 matmuls per 128-vertex tile read the rotated kernel
from a j-duplicated (rotation-deduped, parity-shifted) resident table, then
the norms/argmax/select/relu epilogue runs per tile.
"""

import numpy as np

# Problem constants (hardcoded; kernel.py must be self-contained).
V, NR, NT, CIN, COUT = 50000, 5, 8, 64, 64
NCORES = 8
VPC = V // NCORES            # 6250 vertices per core
TPT = 128                    # vertices per PE tile
NTILES = -(-VPC // TPT)      # 49
VPAD = NTILES * TPT          # 6272
HALFV = V // 2               # 25000 pair-table entries
IJ = NR * NT                 # 40
KP = IJ // 2                 # 20 ij-pairs (matmul chunks)
NQ = 2                       # gather sub-tiles per PE tile
QT = TPT // NQ               # 32 vertices per quarter
NSLOTQ = KP * 3 * QT         # 1920 gather slots per partition-group
NWQ = NSLOTQ * 2             # 3840 weight lanes per group
NIWQ = NSLOTQ // 16          # 120 wrapped idx free dim
ND = NT * COUT               # 512 output cols (r,d)

_CACHE = {}


def build_program(ntiles=NTILES, repeat=1, gather_idxs=NSLOTQ, w_bcast=True):
    """Build the Bacc program for one SPMD core. Returns compiled nc.

    repeat > 1 duplicates the whole tile loop (same inputs/outputs) for
    wall-clock slope timing; the extra passes just overwrite the outputs.
    gather_idxs / w_bcast are timing-ablation knobs (default = real kernel).

    The f32 contraction runs as three accumulating bf16 matmul chains
    (x_hi*k_hi + x_hi*k_lo + x_lo*k_hi, with x = x_hi + x_lo an exact bf16
    split); the dropped x_lo*k_lo term is ~1e-5 relative. 3x512 columns at
    1 cyc/col beats 512 columns of 4-cyc f32.
    """
    import concourse.bass as bass
    import concourse.mybir as mybir
    import concourse.tile as tile
    from concourse import bacc

    f32 = mybir.dt.float32
    f16 = mybir.dt.float16
    i16 = mybir.dt.int16

    nc = bacc.Bacc(
        "TRN2",
        target_bir_lowering=False,
        debug=False,
        enable_asserts=False,
        num_devices=NCORES,
    )
    bf16 = mybir.dt.bfloat16
    sig_d = nc.dram_tensor("sigp", [128, HALFV, 2], f16, kind="ExternalInput")
    kdh_d = nc.dram_tensor("kdh", [128, NR, 16 * COUT], bf16, kind="ExternalInput")
    kdl_d = nc.dram_tensor("kdl", [128, NR, 16 * COUT], bf16, kind="ExternalInput")
    idx_d = nc.dram_tensor(
        "idx16", [ntiles * NQ * 128, NIWQ], i16, kind="ExternalInput"
    )
    wb_d = nc.dram_tensor("wb2", [ntiles, NQ, 2, NSLOTQ], f32, kind="ExternalInput")
    mk_d = nc.dram_tensor(
        "mask8", [ntiles, NQ, 2, NSLOTQ], mybir.dt.uint8, kind="ExternalInput"
    )
    out_d = nc.dram_tensor("out", [ntiles * TPT, COUT], f32, kind="ExternalOutput")

    with tile.TileContext(nc) as tc:
        with (
            tc.tile_pool(name="const", bufs=1) as cpool,
            tc.tile_pool(name="io", bufs=2) as iopool,
            tc.tile_pool(name="w", bufs=2) as wpool,
            tc.tile_pool(name="g", bufs=1) as gpool,
            tc.tile_pool(name="x", bufs=1) as xpool,
            tc.tile_pool(name="xs", bufs=1) as xspool,
            tc.tile_pool(name="epi", bufs=2) as epool,
            tc.tile_pool(name="ps", bufs=4, space="PSUM") as psA,
        ):
            table = cpool.tile([128, HALFV, 2], f16)
            nc.sync.dma_start(out=table[:], in_=sig_d.ap())
            kdh_t = cpool.tile([128, NR, 16 * COUT], bf16)
            nc.sync.dma_start(out=kdh_t[:], in_=kdh_d.ap())
            kdl_t = cpool.tile([128, NR, 16 * COUT], bf16)
            nc.sync.dma_start(out=kdl_t[:], in_=kdl_d.ap())

            for it_rep in range(ntiles * repeat):
                it = it_rep % ntiles
                x_t = xpool.tile([128, KP, TPT], f32, tag="x")
                i4_t = iopool.tile([128, NQ, NIWQ], i16, tag="i")
                nc.gpsimd.dma_start(
                    out=i4_t[:],
                    in_=idx_d.ap()[it * NQ * 128:(it + 1) * NQ * 128, :]
                    .rearrange("(q p) n -> p q n", p=128),
                )
                for q in range(NQ):
                    w_t = wpool.tile([128, NSLOTQ], f32, tag="w")
                    m_t = wpool.tile([128, NSLOTQ], mybir.dt.uint8, tag="m")
                    for parity in range(2):
                        rows = slice(parity * 64, (parity + 1) * 64)
                        nc.gpsimd.dma_start(
                            out=w_t[rows, :],
                            in_=wb_d.ap()[it, q, parity, :]
                            .unsqueeze(0)
                            .to_broadcast([64, NSLOTQ]),
                        )
                        nc.gpsimd.dma_start(
                            out=m_t[rows, :],
                            in_=mk_d.ap()[it, q, parity, :]
                            .unsqueeze(0)
                            .to_broadcast([64, NSLOTQ]),
                        )
                    g_t = gpool.tile([128, NSLOTQ, 2], f16, tag="g")
                    nc.gpsimd.ap_gather(
                        out_ap=g_t[:],
                        in_ap=table[:],
                        idxs_ap=i4_t[:, q, :],
                        channels=128,
                        num_elems=HALFV,
                        d=2,
                        num_idxs=gather_idxs,
                    )
                    # Lane merge: g[:, :, 0] = mask ? g[:, :, 1] : g[:, :, 0]
                    nc.vector.copy_predicated(
                        out=g_t[:, :, 0], mask=m_t[:], data=g_t[:, :, 1]
                    )
                    # xw = w * g_selected in f32, in place over the weights.
                    nc.vector.tensor_tensor(
                        out=w_t[:], in0=w_t[:], in1=g_t[:, :, 0],
                        op=mybir.AluOpType.mult,
                    )
                    # Tap sum over t3 (in place into t=0 slot), second add
                    # lands in the assembled x tile.
                    w4 = w_t[:].rearrange("p (k t v) -> p k t v", k=KP, t=3)
                    nc.vector.tensor_tensor(
                        out=w4[:, :, 0], in0=w4[:, :, 0], in1=w4[:, :, 1],
                        op=mybir.AluOpType.add,
                    )
                    nc.vector.tensor_tensor(
                        out=x_t[:, :, q * QT:(q + 1) * QT],
                        in0=w4[:, :, 0], in1=w4[:, :, 2],
                        op=mybir.AluOpType.add,
                    )

                # Exact bf16 split of x; the matmul runs as three bf16
                # chains, dropping only the ~1e-5 x_lo*k_lo term.
                xh_t = xspool.tile([128, KP, TPT], bf16, tag="xh")
                nc.scalar.copy(out=xh_t[:], in_=x_t[:])
                xl_t = xspool.tile([128, KP, TPT], bf16, tag="xl")
                nc.vector.tensor_tensor(
                    out=xl_t[:], in0=x_t[:], in1=xh_t[:],
                    op=mybir.AluOpType.subtract,
                )
                conv_p = psA.tile([128, ND], f32, tag="conv")
                chains = ((xh_t, kdh_t), (xh_t, kdl_t), (xl_t, kdh_t))
                for ci, (xs_t, kd_t) in enumerate(chains):
                    for k in range(KP):
                        j0 = (2 * k) % NT
                        i0 = (2 * k) // NT
                        nc.tensor.matmul(
                            conv_p[:],
                            lhsT=xs_t[:, k, :],
                            rhs=kd_t[:, i0, j0 * COUT:j0 * COUT + ND],
                            start=(ci == 0 and k == 0),
                            stop=(ci == 2 and k == KP - 1),
                        )

                # Epilogue: norms over d, argmax over r (via is_equal mask),
                # masked-sum select, relu.
                sq_t = epool.tile([128, ND], f32, tag="sq")
                nc.scalar.activation(
                    out=sq_t[:], in_=conv_p[:],
                    func=mybir.ActivationFunctionType.Square,
                )
                norm_t = epool.tile([128, NT], f32, tag="norm")
                nc.vector.tensor_reduce(
                    out=norm_t[:],
                    in_=sq_t[:].rearrange("p (r d) -> p r d", d=COUT),
                    axis=mybir.AxisListType.X,
                    op=mybir.AluOpType.add,
                )
                mx_t = epool.tile([128, 1], f32, tag="mx")
                nc.vector.tensor_reduce(
                    out=mx_t[:], in_=norm_t[:],
                    axis=mybir.AxisListType.X, op=mybir.AluOpType.max,
                )
                mask_t = epool.tile([128, NT], f32, tag="mask")
                nc.vector.tensor_scalar(
                    out=mask_t[:], in0=norm_t[:], scalar1=mx_t[:], scalar2=None,
                    op0=mybir.AluOpType.is_equal,
                )
                msel_t = epool.tile([128, NT, COUT], f32, tag="sq")
                nc.vector.tensor_tensor(
                    out=msel_t[:],
                    in0=conv_p[:].rearrange("p (r d) -> p r d", d=COUT),
                    in1=mask_t[:].unsqueeze(-1).to_broadcast([128, NT, COUT]),
                    op=mybir.AluOpType.mult,
                )
                o_t = epool.tile([128, COUT], f32, tag="o")
                nc.vector.tensor_reduce(
                    out=o_t[:],
                    in_=msel_t[:].rearrange("p r d -> p d r"),
                    axis=mybir.AxisListType.X,
                    op=mybir.AluOpType.add,
                )
                nc.vector.tensor_scalar_max(o_t[:], o_t[:], 0.0)
                nc.sync.dma_start(
                    out=out_d.ap()[it * TPT:(it + 1) * TPT, :], in_=o_t[:]
                )

    nc.compile()
    return nc


def _host_prep(signal, bary_w, bary_idx, kernel, ntiles=NTILES):
    """Build per-core input maps. All host-side numpy, not timed."""
    import ml_dtypes

    bf16 = ml_dtypes.bfloat16
    kern = np.asarray(kernel, np.float32)

    # j-duplicated, parity-shifted rotated kernel:
    # kdup[p, i, jj*64+d] = K[i, (jj + p//64) % NT, p%64, d], jj in [0,16).
    jj16 = np.arange(16)
    parts = []
    for par in range(2):
        kdp = kern[:, (jj16 + par) % NT, :, :]        # [i, jj, c, d]
        parts.append(kdp.transpose(2, 0, 1, 3))        # [c, i, jj, d]
    kdup = np.ascontiguousarray(
        np.concatenate(parts, axis=0).reshape(128, NR, 16 * COUT)
    )
    kdh = kdup.astype(bf16)
    kdl = (kdup - kdh.astype(np.float32)).astype(bf16)

    # Signal pair table: table[p, e] = (sig[e, c], sig[e+HALFV, c]), c = p%64.
    sb = np.asarray(signal).astype(np.float16)         # [V, 64]
    sigp = np.empty((128, HALFV, 2), np.float16)
    sigp[:, :, 0] = np.tile(sb[:HALFV].T, (2, 1))
    sigp[:, :, 1] = np.tile(sb[HALFV:].T, (2, 1))

    vpad = ntiles * TPT
    idxf = np.asarray(bary_idx).reshape(V, IJ, 3)
    wff = np.asarray(bary_w).reshape(V, IJ, 3).astype(np.float32)
    in_maps = []
    for c in range(NCORES):
        sl = slice(c * VPC, (c + 1) * VPC)
        n = min(VPC, vpad)
        idx = np.zeros((vpad, IJ, 3), np.int32)
        idx[:n] = idxf[sl][:n]
        w = np.zeros((vpad, IJ, 3), np.float32)
        w[:n] = wff[sl][:n]
        # [t, q, v32, k, parity, t3] -> [t, q, parity, k, t3, v32]
        idx_r = idx.reshape(ntiles, NQ, QT, KP, 2, 3).transpose(0, 1, 4, 3, 5, 2)
        w_r = w.reshape(ntiles, NQ, QT, KP, 2, 3).transpose(0, 1, 4, 3, 5, 2)
        e = (idx_r % HALFV).astype(np.int16)
        lane = idx_r // HALFV
        wb2 = np.ascontiguousarray(
            w_r.reshape(ntiles, NQ, 2, NSLOTQ), np.float32
        )
        mask8 = np.ascontiguousarray(
            lane.astype(np.uint8).reshape(ntiles, NQ, 2, NSLOTQ)
        )
        # Wrapped idx: list position i at [i%16, i//16]; 4 copies per half.
        el = e.reshape(ntiles, NQ, 2, NIWQ, 16)
        wr = np.swapaxes(el, -1, -2)                   # [t, q, parity, 16, NIWQ]
        idx16 = np.empty((ntiles, NQ, 128, NIWQ), np.int16)
        idx16[:, :, 0:64] = np.tile(wr[:, :, 0], (1, 1, 4, 1))
        idx16[:, :, 64:128] = np.tile(wr[:, :, 1], (1, 1, 4, 1))
        in_maps.append({
            "sigp": sigp,
            "kdh": kdh,
            "kdl": kdl,
            "idx16": np.ascontiguousarray(
                idx16.reshape(ntiles * NQ * 128, NIWQ)
            ),
            "wb2": wb2,
            "mask8": mask8,
        })
    return in_maps


def kernel(signal, bary_w, bary_idx, kernel):
    from concourse.bass_utils import run_bass_kernel_spmd

    if "nc" not in _CACHE:
        _CACHE["nc"] = build_program()
    nc = _CACHE["nc"]
    in_maps = _host_prep(signal, bary_w, bary_idx, kernel)
    res = run_bass_kernel_spmd(nc, in_maps, core_ids=list(range(NCORES)))
    out = np.concatenate(
        [res.results[c]["out"][:VPC] for c in range(NCORES)], axis=0
    )
    return out.astype(np.float32)
